# revision 1
# baseline (speedup 1.0000x reference)
"""Event-driven FFN kernel for Trainium2 (8 NeuronCores, data-parallel).

Reference computation (per row r of x[32768, 512]):
    mask[r] = any(|x[r, :]| > 0.01)
    y[r, :] = mask[r] * (relu(x[r, :] @ w1 + b1) @ w2 + b2)

Sharding: rows (B*T*S = 32768) split evenly across 8 cores; FFN weights
replicated.  Per core: 4096 rows, processed in 8 blocks of 512 rows.

Per-block dataflow on one core (512 rows per block):
  - DMA x block natural layout [128p, 4rs, 512d]
  - abs-max over d per row -> spike mask (VectorE reduce + is_gt)
  - PE transpose (identity matmul, f32r) -> xT [128d_in, 4dc, 512r]
  - mm1 per f-chunk (16): psum_h[f,r] += w1[dc,f].T @ xT[dc,r]  (4 MMs)
    ReLU+b1 on ScalarE -> hT sbuf [128f_in, 16fc, 512r] (f32r, rounded)
  - mm2 two f-chunks behind mm1 (software pipeline): psum_y[rt] +=
    hT[:,fc,rt].T @ w2[fc,:] -> natural-layout y rows in PSUM (4 banks
    live across the f loop; 5-slot pool so slot reuse never stalls PE)
  - epilogue: yb = b2*mask precomputed off-path; one fused VectorE op per
    row-subtile yout = psy*mask + yb, then DMA out per row-subtile

Scheduling notes:
  - Block b+1's x load + PE transposes are emitted mid-way through block
    b's f-loop so PE never stalls on the DVE xT copies.
  - Weights stream in chunks in first-use order; block 0 defers all mm2s
    past its mm1 phase so they aren't gated on the still-streaming w2.
  - All matmuls use float32r (PE truncates to ~FP22, full 1 cycle/row
    streaming rate; plain float32 runs 4 passes = 4x slower).  Rel err vs
    the f32 reference is ~2e-4.
  - Built on bacc.Bacc: finalize() legalizes multi-sem-wait instructions
    (TRN2 engines accept one sem wait per instruction).

TimelineSim (cost model): ~241.9 us/core end-to-end one-shot; steady-state
marginal pass is ~228.8 us = PE fully saturated (218 us matmuls at the
1 cycle/row f32r rate + 10 us PE transposes).  Remaining one-shot overhead
is the DMA-bandwidth-bound startup stream (~3 us; 2 MB must land before
the first mm1 can run), the Tile entry barrier (~1.4 us) and exit drain
(~4.3 us; the last block groups mm2 by row-subtile so only one epilogue
trails the final matmul).  PSUM: 3 staging banks (transpose + mm1
accumulator, shared tag) + 4 y-accumulator banks + 1 warmup bank.
"""

import numpy as np

N_CORES = 8
ROWS_TOTAL = 32768  # 4 * 16 * 512
ROWS_PER_CORE = ROWS_TOTAL // N_CORES  # 4096
D = 512
F = 2048
R_BLOCK = 512
N_BLOCKS = ROWS_PER_CORE // R_BLOCK  # 8
P = 128
DC = D // P  # 4 d-chunks
FC = F // P  # 16 f-chunks
RT = R_BLOCK // P  # 4 row-subtiles per block
THRESHOLD = 0.01

_CACHE = {}


def _build_program(repeat=1):
    import concourse.mybir as mybir
    import concourse.tile as tile
    from concourse import bacc
    from concourse.masks import make_identity

    f32 = mybir.dt.float32
    f32r = mybir.dt.float32r
    # Bacc (not plain Bass): finalize() runs the wait-splitting legalization
    # (generate_event_semaphores) required by TRN2's 1-wait-per-instruction
    # hardware limit.
    nc = bacc.Bacc()

    x = nc.declare_dram_parameter("x", [ROWS_PER_CORE, D], f32, isOutput=False)
    w1 = nc.declare_dram_parameter("w1", [D, F], f32, isOutput=False)
    b1 = nc.declare_dram_parameter("b1", [F], f32, isOutput=False)
    w2 = nc.declare_dram_parameter("w2", [F, D], f32, isOutput=False)
    b2 = nc.declare_dram_parameter("b2", [D], f32, isOutput=False)
    y = nc.declare_dram_parameter("y", [ROWS_PER_CORE, D], f32, isOutput=True)

    n_iter = N_BLOCKS * repeat

    with tile.TileContext(nc) as tc:
        with (
            tc.tile_pool(name="const", bufs=1) as const,
            tc.tile_pool(name="xin", bufs=2) as xin_pool,
            tc.tile_pool(name="xt", bufs=2) as xt_pool,
            tc.tile_pool(name="h", bufs=2) as h_pool,
            tc.tile_pool(name="out", bufs=2) as out_pool,
            tc.tile_pool(name="mask", bufs=2) as mask_pool,
            tc.tile_pool(name="stage", bufs=3, space="PSUM") as stage_pool,
            tc.tile_pool(name="py", bufs=4, space="PSUM") as py_pool,
            tc.tile_pool(name="warm", bufs=1, space="PSUM") as warm_pool,
        ):
            # Replicated parameters.  Chunked so the first matmuls can start
            # as soon as their slice arrives instead of behind 8 MB of DMA.
            w1s = const.tile([P, DC, F], f32r)  # [p, dc, f] <- w1[dc*128+p, f]
            w2s = const.tile([P, FC, D], f32r)  # [p, fc, d] <- w2[fc*128+p, d]
            b1s = const.tile([P, FC], f32)  # [p, fc] <- b1[fc*128+p]
            b2s = const.tile([P, D], f32)  # b2 replicated to all partitions
            ident = const.tile([P, P], f32r)

            w1r = w1.rearrange("(dc p) f -> p dc f", p=P).bitcast(f32r)
            w2r = w2.rearrange("(fc p) d -> p fc d", p=P).bitcast(f32r)

            def load_x(blk):
                rows = x[blk * R_BLOCK : (blk + 1) * R_BLOCK, :]
                src_ap = rows.rearrange("(rs p) d -> p rs d", p=P).bitcast(f32r)
                xn = xin_pool.tile([P, RT, D], f32r, name="xn")
                nc.sync.dma_start(xn[:], src_ap)
                return xn

            def mask_and_transpose(xn):
                # Transpose x -> xT [d_inner, dc, r] via PE (f32r: 1.5 c/row).
                # Emitted before the mask ops: the DVE psum->SBUF copies gate
                # the next mm1, the mask is only needed at the epilogue.
                xT = xt_pool.tile([P, DC, R_BLOCK], f32r, name="xT")
                for dc in range(DC):
                    # Grouped by d-chunk (not row-subtile): one DVE copy then
                    # delivers a complete mm1 rhs, so the first matmul starts
                    # as soon as the first chunk is staged.
                    pt = stage_pool.tile(
                        [P, RT, P], f32r, name="pt", tag="stage"
                    )
                    for rs in range(RT):
                        nc.tensor.transpose(
                            pt[:, rs, :],
                            xn[:, rs, dc * P : (dc + 1) * P],
                            ident[:],
                        )
                    last_copy = nc.vector.tensor_copy(xT[:, dc, :], pt[:])

                # Spike mask: 1.0 where max_d |x| > threshold else 0.0.
                amax = mask_pool.tile([P, RT], f32, name="amax")
                reduce_inst = nc.vector.tensor_reduce(
                    amax[:],
                    xn.bitcast(f32)[:],
                    axis=mybir.AxisListType.X,
                    op=mybir.AluOpType.max,
                    apply_absolute_value=True,
                )
                # Scheduling-only edge: keep the 2.2us reduce behind the xT
                # copies on DVE -- the copies gate the next mm1, the mask is
                # not needed until the epilogue.
                tile.add_dep_helper(
                    reduce_inst.ins, last_copy.ins, sync=False,
                    reason="mask reduce after xT copies",
                )
                mask = mask_pool.tile([P, RT], f32, name="mask")
                nc.vector.tensor_scalar(
                    mask[:], amax[:], THRESHOLD, None, op0=mybir.AluOpType.is_gt
                )
                return {"xT": xT, "mask": mask}

            # PE clock warm-up: the PE ramps to full clock only after ~3us
            # of sustained activity (HAM gate).  Burn the ramp on
            # dependency-free dummy matmuls (memset-fed, bf16) during the
            # dead window while x block 0 / w1 stream in, so the real
            # transposes+matmuls start at full rate.
            bf16 = mybir.dt.bfloat16
            wsrc = const.tile([P, D], bf16)
            nc.vector.memset(wsrc[:], 0.0)
            wdummy = warm_pool.tile([P, D], f32)
            for _ in range(10):
                nc.tensor.matmul(
                    wdummy[:], wsrc[:, 0:P], wsrc[:], start=True, stop=True
                )

            # --- startup: stream in first-use order.  Block 0's mm1 phase
            # only needs w1 (streamed in quarters just ahead of use); w2
            # chunks follow and land before block 0's (deferred) mm2 phase.
            xn0 = load_x(0)
            nc.sync.dma_start(w1s[:, :, 0:512], w1r[:, :, 0:512])
            nc.sync.dma_start(b1s[:], b1.rearrange("(fc p) -> p fc", p=P))
            nc.sync.dma_start(w1s[:, :, 512:1024], w1r[:, :, 512:1024])
            # Build identity in f32 scratch, then copy (=round) into the
            # f32r tile the transposes consume (BIR verifier requirement).
            ident_f32 = const.tile([P, P], f32)
            make_identity(nc, ident_f32)
            nc.vector.tensor_copy(ident[:], ident_f32[:])
            cur = mask_and_transpose(xn0)
            nc.sync.dma_start(w1s[:, :, 1024:1536], w1r[:, :, 1024:1536])
            nc.sync.dma_start(w1s[:, :, 1536:2048], w1r[:, :, 1536:2048])
            # x(1) right after w1 (its transposes run early in block 0's
            # deferred-mm2 phase), then w2 chunks just ahead of their mm2s.
            xn_next = load_x(1 % N_BLOCKS) if n_iter > 1 else None
            for wc in range(4):
                nc.sync.dma_start(
                    w2s[:, 4 * wc : 4 * (wc + 1), :],
                    w2r[:, 4 * wc : 4 * (wc + 1), :],
                )
            nc.sync.dma_start(b2s[:], b2[None, :].to_broadcast([P, D]))

            for it in range(n_iter):
                blk = it % N_BLOCKS
                xT, mask = cur["xT"], cur["mask"]

                hs = h_pool.tile([P, FC, R_BLOCK], f32r, name="hs")  # h^T
                psy = [
                    py_pool.tile([P, D], f32, name=f"psy{rt}", tag="psy")
                    for rt in range(RT)
                ]
                nxt = None
                # b2 * mask per row-subtile, off the critical path (feeds
                # the fused single-op epilogue below).
                yb = out_pool.tile([P, RT, D], f32, name="yb")
                for rt in range(RT):
                    nc.vector.tensor_scalar_mul(
                        yb[:, rt, :], b2s[:], mask[:, rt : rt + 1]
                    )

                def mm2(fc):
                    for rt in range(RT):
                        nc.tensor.matmul(
                            psy[rt][:],
                            hs[:, fc, rt * P : (rt + 1) * P],
                            w2s[:, fc, :],
                            start=(fc == 0),
                            stop=(fc == FC - 1),
                        )

                # Software-pipelined: mm2 runs one f-chunk behind mm1/relu
                # so PE never waits on ScalarE at block boundaries.  Block 0
                # instead defers ALL mm2s past the mm1 phase so they aren't
                # stuck behind the still-streaming w2 (PE does w1-only work
                # while w2 lands).
                mm2_lag = FC if (it == 0 or it == n_iter - 1) else 2
                for fc in range(FC):
                    ph = stage_pool.tile(
                        [P, R_BLOCK], f32, name="ph", tag="stage"
                    )
                    for dc in range(DC):
                        nc.tensor.matmul(
                            ph[:],
                            w1s[:, dc, fc * P : (fc + 1) * P],
                            xT[:, dc, :],
                            start=(dc == 0),
                            stop=(dc == DC - 1),
                        )
                    nc.scalar.activation(
                        hs[:, fc, :],
                        ph[:],
                        mybir.ActivationFunctionType.Relu,
                        bias=b1s[:, fc : fc + 1],
                    )
                    if fc >= mm2_lag:
                        mm2(fc - mm2_lag)
                    # Prefetch: x DMA for block it+2 early (fc==1), next
                    # block's transposes mid-way so PE never stalls.  For
                    # block 0 the transposes wait until fc==15 (x(1) is still
                    # behind w1 in the DMA stream at fc==7).
                    if fc == 1 and it + 2 < n_iter:
                        xn_next2 = load_x((it + 2) % N_BLOCKS)
                    if fc == (15 if it == 0 else 7) and it + 1 < n_iter:
                        nxt = mask_and_transpose(xn_next)
                        xn_next = xn_next2 if it + 2 < n_iter else None
                # Epilogue: yout = psy*mask + b2*mask, one fused DVE op per
                # row-subtile (psy bank freed after a single op).
                yout = out_pool.tile([P, RT, D], f32, name="yout")

                def epilogue(rt):
                    nc.vector.scalar_tensor_tensor(
                        yout[:, rt, :],
                        psy[rt][:],
                        mask[:, rt : rt + 1],
                        yb[:, rt, :],
                        op0=mybir.AluOpType.mult,
                        op1=mybir.AluOpType.add,
                    )
                    out_rows = y[
                        blk * R_BLOCK + rt * P : blk * R_BLOCK + (rt + 1) * P, :
                    ]
                    nc.sync.dma_start(out_rows, yout[:, rt, :])

                if it == n_iter - 1 and it != 0:
                    # Last block: group the remaining mm2s by row-subtile so
                    # each subtile's epilogue + store overlaps the next
                    # subtile's matmuls; only rt3's epilogue trails the final
                    # PE op before the kernel drain.
                    done = FC - mm2_lag
                    for rt in range(RT):
                        for fc in range(done, FC):
                            nc.tensor.matmul(
                                psy[rt][:],
                                hs[:, fc, rt * P : (rt + 1) * P],
                                w2s[:, fc, :],
                                start=(fc == 0),
                                stop=(fc == FC - 1),
                            )
                        epilogue(rt)
                else:
                    for fc in range(FC - mm2_lag, FC):
                        mm2(fc)
                    for rt in range(RT):
                        epilogue(rt)
                cur = nxt

    nc.finalize()
    return nc


def _get_program():
    if "nc" not in _CACHE:
        _CACHE["nc"] = _build_program()
    return _CACHE["nc"]


def kernel(x, w1, b1, w2, b2, _trace=False):
    from concourse.bass_utils import run_bass_kernel_spmd

    x = np.ascontiguousarray(np.asarray(x, dtype=np.float32))
    w1 = np.ascontiguousarray(np.asarray(w1, dtype=np.float32))
    b1 = np.ascontiguousarray(np.asarray(b1, dtype=np.float32))
    w2 = np.ascontiguousarray(np.asarray(w2, dtype=np.float32))
    b2 = np.ascontiguousarray(np.asarray(b2, dtype=np.float32))

    B, T, S, Dd = x.shape
    xf = x.reshape(-1, Dd)
    shards = np.split(xf, N_CORES, axis=0)
    in_maps = [
        {"x": s, "w1": w1, "b1": b1, "w2": w2, "b2": b2} for s in shards
    ]

    nc = _get_program()
    # The axon-tunneled devices occasionally throw a transient
    # NRT_EXEC_UNIT_UNRECOVERABLE; a fresh attempt succeeds.
    last_err = None
    for _attempt in range(3):
        try:
            res = run_bass_kernel_spmd(
                nc, in_maps, list(range(N_CORES)), trace=_trace
            )
            break
        except Exception as e:  # noqa: BLE001 - retry transient device faults
            last_err = e
            if "UNRECOVERABLE" not in str(e) and "UNAVAILABLE" not in str(e):
                raise
    else:
        raise last_err
    yf = np.concatenate([r["y"] for r in res.results], axis=0)
    out = yf.reshape(B, T, S, Dd).astype(np.float32)
    if _trace:
        return out, res
    return out



# revision 2
# speedup vs baseline: 1.0241x; 1.0241x over previous
"""Event-driven FFN kernel for Trainium2 (8 NeuronCores, data-parallel).

Reference computation (per row r of x[32768, 512]):
    mask[r] = any(|x[r, :]| > 0.01)
    y[r, :] = mask[r] * (relu(x[r, :] @ w1 + b1) @ w2 + b2)

Sharding: rows (B*T*S = 32768) split evenly across 8 cores; FFN weights
replicated.  Per core: 4096 rows, processed in 8 blocks of 512 rows.

Key design points (v2, fp16 + DMA-xbar transpose):
  - x/w1/w2 are cast to fp16 on the HOST (numpy, free) before upload.
    fp16 matmuls run at the same 1 cycle/row PE rate as f32r but (a) the
    x transpose moves off the PE onto the DMA xbar transpose engine
    (InstDmaTransposeAnt, 16-bit only), and (b) x/weight DMA bytes halve.
    End-to-end rel err vs the f32 reference is ~6e-4 (fp16 has a 10-bit
    mantissa; accumulation stays f32 in PSUM).
  - Per block: one DMA-transpose DRAM->SBUF lands x directly as
    xT [128 d_inner, dc, 512 r] (cost-model 14ns per 16x128 tile =
    1.79us/block, overlapped); a second natural-layout load feeds the
    spike-mask reduce on DVE.  The PE does nothing but matmuls:
    8 blocks x (mm1 64 + mm2 64) x 512 rows = 524288 cycles = 218.5us.
  - mm1 per f-chunk (16): psum_h[f,r] += w1[dc,f].T @ xT[dc,r] (4 MMs),
    ReLU+b1 on ScalarE -> hT sbuf fp16 [128f_in, 16fc, 512r].
  - mm2 two f-chunks behind mm1 (software pipeline): psum_y[rt] +=
    hT[:,fc,rt].T @ w2[fc,:]; block 0 defers all mm2s past its mm1 phase
    so they aren't gated on the still-streaming w2.
  - Epilogue: yb = b2*mask precomputed off-path; one fused DVE op per
    row-subtile yout = psy*mask + yb, then DMA out per row-subtile.
    Last block groups mm2 by row-subtile so only rt3's epilogue trails
    the final matmul.
  - PE clock warm-up (HAM gate) burned on dummy matmuls during the
    startup DMA window.
  - Built on bacc.Bacc: finalize() legalizes multi-sem-wait instructions.
"""

import numpy as np

N_CORES = 8
ROWS_TOTAL = 32768  # 4 * 16 * 512
ROWS_PER_CORE = ROWS_TOTAL // N_CORES  # 4096
D = 512
F = 2048
R_BLOCK = 512
N_BLOCKS = ROWS_PER_CORE // R_BLOCK  # 8
P = 128
DC = D // P  # 4 d-chunks
FC = F // P  # 16 f-chunks
RT = R_BLOCK // P  # 4 row-subtiles per block
THRESHOLD = 0.01

_CACHE = {}


def _build_program(repeat=1):
    import concourse.mybir as mybir
    import concourse.tile as tile
    from concourse import bacc

    f32 = mybir.dt.float32
    f16 = mybir.dt.float16
    nc = bacc.Bacc()

    x = nc.declare_dram_parameter("x", [ROWS_PER_CORE, D], f16, isOutput=False)
    w1 = nc.declare_dram_parameter("w1", [D, F], f16, isOutput=False)
    b1 = nc.declare_dram_parameter("b1", [F], f32, isOutput=False)
    w2 = nc.declare_dram_parameter("w2", [F, D], f16, isOutput=False)
    b2 = nc.declare_dram_parameter("b2", [D], f32, isOutput=False)
    y = nc.declare_dram_parameter("y", [ROWS_PER_CORE, D], f32, isOutput=True)

    n_iter = N_BLOCKS * repeat

    with tile.TileContext(nc) as tc:
        with (
            tc.tile_pool(name="const", bufs=1) as const,
            tc.tile_pool(name="xin", bufs=2) as xin_pool,
            tc.tile_pool(name="xt", bufs=2) as xt_pool,
            tc.tile_pool(name="h", bufs=2) as h_pool,
            tc.tile_pool(name="out", bufs=2) as out_pool,
            tc.tile_pool(name="mask", bufs=2) as mask_pool,
            tc.tile_pool(name="stage", bufs=3, space="PSUM") as stage_pool,
            tc.tile_pool(name="py", bufs=4, space="PSUM") as py_pool,
            tc.tile_pool(name="warm", bufs=1, space="PSUM") as warm_pool,
        ):
            # Replicated parameters, chunked so the first matmuls can start
            # as soon as their slice arrives.
            w1s = const.tile([P, DC, F], f16)  # [p, dc, f] <- w1[dc*128+p, f]
            w2s = const.tile([P, FC, D], f16)  # [p, fc, d] <- w2[fc*128+p, d]
            b1s = const.tile([P, FC], f32)  # [p, fc] <- b1[fc*128+p]
            b2s = const.tile([P, D], f32)  # b2 replicated to all partitions

            w1r = w1.rearrange("(dc p) f -> p dc f", p=P)
            w2r = w2.rearrange("(fc p) d -> p fc d", p=P)

            def load_xT(blk):
                # DMA xbar transpose straight from DRAM:
                # out[d_in, dc, r] = x[blk*512 + r, dc*128 + d_in]
                xT = xt_pool.tile([P, DC, R_BLOCK], f16, name="xT")
                rows = x[blk * R_BLOCK : (blk + 1) * R_BLOCK, :]
                nc.sync.dma_start(xT[:], rows, transpose=True)
                return xT

            def load_xn(blk):
                rows = x[blk * R_BLOCK : (blk + 1) * R_BLOCK, :]
                src_ap = rows.rearrange("(rs p) d -> p rs d", p=P)
                xn = xin_pool.tile([P, RT, D], f16, name="xn")
                nc.sync.dma_start(xn[:], src_ap)
                return xn

            def make_mask(xn):
                # Spike mask: 1.0 where max_d |x| > threshold else 0.0.
                amax = mask_pool.tile([P, RT], f32, name="amax")
                nc.vector.tensor_reduce(
                    amax[:],
                    xn[:],
                    axis=mybir.AxisListType.X,
                    op=mybir.AluOpType.max,
                    apply_absolute_value=True,
                )
                mask = mask_pool.tile([P, RT], f32, name="mask")
                nc.vector.tensor_scalar(
                    mask[:], amax[:], THRESHOLD, None, op0=mybir.AluOpType.is_gt
                )
                return mask

            # PE clock warm-up: the PE ramps to full clock only after ~3us
            # of sustained activity (HAM gate).  Burn the ramp on
            # dependency-free dummy matmuls during the startup DMA window.
            bf16 = mybir.dt.bfloat16
            wsrc = const.tile([P, D], bf16)
            nc.vector.memset(wsrc[:], 0.0)
            wdummy = warm_pool.tile([P, D], f32)
            for _ in range(10):
                nc.tensor.matmul(
                    wdummy[:], wsrc[:, 0:P], wsrc[:], start=True, stop=True
                )

            # --- startup: stream in first-use order.  Block 0's mm1 phase
            # only needs xT(0) + w1 (streamed in quarters just ahead of
            # use); w2 chunks follow and land before block 0's (deferred)
            # mm2 phase.
            xT0 = load_xT(0)
            nc.sync.dma_start(w1s[:, :, 0:512], w1r[:, :, 0:512])
            xn0 = load_xn(0)
            nc.sync.dma_start(b1s[:], b1.rearrange("(fc p) -> p fc", p=P))
            nc.sync.dma_start(w1s[:, :, 512:1024], w1r[:, :, 512:1024])
            cur = {"xT": xT0, "mask": make_mask(xn0)}
            nc.sync.dma_start(w1s[:, :, 1024:1536], w1r[:, :, 1024:1536])
            nc.sync.dma_start(w1s[:, :, 1536:2048], w1r[:, :, 1536:2048])
            if n_iter > 1:
                nxt_pending = {"xT": load_xT(1), "xn": load_xn(1)}
            else:
                nxt_pending = None
            for wc in range(4):
                nc.sync.dma_start(
                    w2s[:, 4 * wc : 4 * (wc + 1), :],
                    w2r[:, 4 * wc : 4 * (wc + 1), :],
                )
            nc.sync.dma_start(b2s[:], b2[None, :].to_broadcast([P, D]))

            for it in range(n_iter):
                blk = it % N_BLOCKS
                xT, mask = cur["xT"], cur["mask"]

                hs = h_pool.tile([P, FC, R_BLOCK], f16, name="hs")  # h^T
                psy = [
                    py_pool.tile([P, D], f32, name=f"psy{rt}", tag="psy")
                    for rt in range(RT)
                ]
                # b2 * mask per row-subtile, off the critical path (feeds
                # the fused single-op epilogue below).
                yb = out_pool.tile([P, RT, D], f32, name="yb")
                for rt in range(RT):
                    nc.vector.tensor_scalar_mul(
                        yb[:, rt, :], b2s[:], mask[:, rt : rt + 1]
                    )

                def mm2(fc):
                    for rt in range(RT):
                        nc.tensor.matmul(
                            psy[rt][:],
                            hs[:, fc, rt * P : (rt + 1) * P],
                            w2s[:, fc, :],
                            start=(fc == 0),
                            stop=(fc == FC - 1),
                        )

                # Software-pipelined: mm2 runs two f-chunks behind mm1/relu
                # so PE never waits on ScalarE at block boundaries.  Block 0
                # instead defers ALL mm2s past the mm1 phase so they aren't
                # stuck behind the still-streaming w2.
                mm2_lag = FC if (it == 0 or it == n_iter - 1) else 2
                for fc in range(FC):
                    ph = stage_pool.tile(
                        [P, R_BLOCK], f32, name="ph", tag="stage"
                    )
                    for dc in range(DC):
                        nc.tensor.matmul(
                            ph[:],
                            w1s[:, dc, fc * P : (fc + 1) * P],
                            xT[:, dc, :],
                            start=(dc == 0),
                            stop=(dc == DC - 1),
                        )
                    nc.scalar.activation(
                        hs[:, fc, :],
                        ph[:],
                        mybir.ActivationFunctionType.Relu,
                        bias=b1s[:, fc : fc + 1],
                    )
                    if fc >= mm2_lag:
                        mm2(fc - mm2_lag)
                    # Prefetch block it+2's x mid-way through this block;
                    # compute block it+1's mask once its natural load is in.
                    if fc == 1 and it + 2 < n_iter:
                        nxt2_pending = {
                            "xT": load_xT((it + 2) % N_BLOCKS),
                            "xn": load_xn((it + 2) % N_BLOCKS),
                        }
                    if fc == 7 and it + 1 < n_iter:
                        nxt = {
                            "xT": nxt_pending["xT"],
                            "mask": make_mask(nxt_pending["xn"]),
                        }
                        nxt_pending = (
                            nxt2_pending if it + 2 < n_iter else None
                        )
                # Epilogue: yout = psy*mask + b2*mask, one fused DVE op per
                # row-subtile (psy bank freed after a single op).
                yout = out_pool.tile([P, RT, D], f32, name="yout")

                def epilogue(rt):
                    nc.vector.scalar_tensor_tensor(
                        yout[:, rt, :],
                        psy[rt][:],
                        mask[:, rt : rt + 1],
                        yb[:, rt, :],
                        op0=mybir.AluOpType.mult,
                        op1=mybir.AluOpType.add,
                    )
                    out_rows = y[
                        blk * R_BLOCK + rt * P : blk * R_BLOCK + (rt + 1) * P, :
                    ]
                    nc.sync.dma_start(out_rows, yout[:, rt, :])

                if it == n_iter - 1 and it != 0:
                    # Last block: group the remaining mm2s by row-subtile so
                    # each subtile's epilogue + store overlaps the next
                    # subtile's matmuls; only rt3's epilogue trails the final
                    # PE op before the kernel drain.
                    done = FC - mm2_lag
                    for rt in range(RT):
                        for fc in range(done, FC):
                            nc.tensor.matmul(
                                psy[rt][:],
                                hs[:, fc, rt * P : (rt + 1) * P],
                                w2s[:, fc, :],
                                start=(fc == 0),
                                stop=(fc == FC - 1),
                            )
                        epilogue(rt)
                else:
                    for fc in range(FC - mm2_lag, FC):
                        mm2(fc)
                    for rt in range(RT):
                        epilogue(rt)
                if it + 1 < n_iter:
                    cur = nxt

    nc.finalize()
    return nc


def _get_program():
    if "nc" not in _CACHE:
        _CACHE["nc"] = _build_program()
    return _CACHE["nc"]


def kernel(x, w1, b1, w2, b2, _trace=False):
    from concourse.bass_utils import run_bass_kernel_spmd

    # fp16 casts are host-side (free); accumulation on-device stays f32.
    x = np.ascontiguousarray(np.asarray(x, dtype=np.float32))
    x16 = x.astype(np.float16)
    w1h = np.ascontiguousarray(np.asarray(w1, dtype=np.float16))
    b1 = np.ascontiguousarray(np.asarray(b1, dtype=np.float32))
    w2h = np.ascontiguousarray(np.asarray(w2, dtype=np.float16))
    b2 = np.ascontiguousarray(np.asarray(b2, dtype=np.float32))

    B, T, S, Dd = x16.shape
    xf = x16.reshape(-1, Dd)
    shards = np.split(xf, N_CORES, axis=0)
    in_maps = [
        {"x": s, "w1": w1h, "b1": b1, "w2": w2h, "b2": b2} for s in shards
    ]

    nc = _get_program()
    # The axon-tunneled devices occasionally throw a transient
    # NRT_EXEC_UNIT_UNRECOVERABLE; a fresh attempt succeeds.
    last_err = None
    for _attempt in range(3):
        try:
            res = run_bass_kernel_spmd(
                nc, in_maps, list(range(N_CORES)), trace=_trace
            )
            break
        except Exception as e:  # noqa: BLE001 - retry transient device faults
            last_err = e
            if "UNRECOVERABLE" not in str(e) and "UNAVAILABLE" not in str(e):
                raise
    else:
        raise last_err
    yf = np.concatenate([r["y"] for r in res.results], axis=0)
    out = yf.reshape(B, T, S, Dd).astype(np.float32)
    if _trace:
        return out, res
    return out


# revision 19
# speedup vs baseline: 1.0592x; 1.0343x over previous
"""Event-driven FFN kernel for Trainium2 (8 NeuronCores, data-parallel).

Reference computation (per row r of x[32768, 512]):
    mask[r] = any(|x[r, :]| > 0.01)
    y[r, :] = mask[r] * (relu(x[r, :] @ w1 + b1) @ w2 + b2)

Sharding: rows (B*T*S = 32768) split evenly across 8 cores; FFN weights
replicated.  Per core: 4096 rows, processed in 8 blocks of 512 rows.

Key design points (v2, fp16 + DMA-xbar transpose):
  - x/w1/w2 are cast to fp16 on the HOST (numpy, free) before upload.
    fp16 matmuls run at the same 1 cycle/row PE rate as f32r but (a) the
    x transpose moves off the PE onto the DMA xbar transpose engine
    (InstDmaTransposeAnt, 16-bit only), and (b) x/weight DMA bytes halve.
    End-to-end rel err vs the f32 reference is ~6e-4 (fp16 has a 10-bit
    mantissa; accumulation stays f32 in PSUM).
  - Per block: one DMA-transpose DRAM->SBUF lands x directly as
    xT [128 d_inner, dc, 512 r] (cost-model 14ns per 16x128 tile =
    1.79us/block, overlapped); a second natural-layout load feeds the
    spike-mask reduce on DVE.  The PE does nothing but matmuls:
    8 blocks x (mm1 64 + mm2 64) x 512 rows = 524288 cycles = 218.5us.
  - mm1 per f-chunk (16): psum_h[f,r] += w1[dc,f].T @ xT[dc,r] (4 MMs),
    ReLU+b1 on ScalarE -> hT sbuf fp16 [128f_in, 16fc, 512r].
  - mm2 two f-chunks behind mm1 (software pipeline): psum_y[rt] +=
    hT[:,fc,rt].T @ w2[fc,:]; block 0 defers all mm2s past its mm1 phase
    so they aren't gated on the still-streaming w2.
  - Epilogue: yb = b2*mask precomputed off-path; one fused DVE op per
    row-subtile yout = psy*mask + yb, then DMA out per row-subtile.
    Last block groups mm2 by row-subtile so only rt3's epilogue trails
    the final matmul.
  - PE clock warm-up (HAM gate) burned on dummy matmuls during the
    startup DMA window.
  - Built on bacc.Bacc: finalize() legalizes multi-sem-wait instructions.
"""

import numpy as np

N_CORES = 8
ROWS_TOTAL = 32768  # 4 * 16 * 512
ROWS_PER_CORE = ROWS_TOTAL // N_CORES  # 4096
D = 512
F = 2048
R_BLOCK = 512
N_BLOCKS = ROWS_PER_CORE // R_BLOCK  # 8
P = 128
DC = D // P  # 4 d-chunks
FC = F // P  # 16 f-chunks
RT = R_BLOCK // P  # 4 row-subtiles per block
THRESHOLD = 0.01

_CACHE = {}


def _build_program(repeat=1):
    import concourse.mybir as mybir
    import concourse.tile as tile
    from concourse import bacc

    f32 = mybir.dt.float32
    f16 = mybir.dt.float16
    nc = bacc.Bacc()

    x = nc.declare_dram_parameter("x", [ROWS_PER_CORE, D], f16, isOutput=False)
    w1 = nc.declare_dram_parameter("w1", [D, F], f16, isOutput=False)
    b1 = nc.declare_dram_parameter("b1", [F], f32, isOutput=False)
    w2 = nc.declare_dram_parameter("w2", [F, D], f16, isOutput=False)
    b2 = nc.declare_dram_parameter("b2", [D], f32, isOutput=False)
    y = nc.declare_dram_parameter("y", [ROWS_PER_CORE, D], f32, isOutput=True)
    identity = nc.declare_dram_parameter(
        "identity", [P, P], f16, isOutput=False
    )

    n_iter = N_BLOCKS * repeat

    with tile.TileContext(nc) as tc:
        with (
            tc.tile_pool(name="const", bufs=1) as const,
            tc.tile_pool(name="xin", bufs=2) as xin_pool,
            tc.tile_pool(name="xt", bufs=2) as xt_pool,
            tc.tile_pool(name="h", bufs=2) as h_pool,
            tc.tile_pool(name="out", bufs=2) as out_pool,
            tc.tile_pool(name="mask", bufs=2) as mask_pool,
            tc.tile_pool(name="stage", bufs=3, space="PSUM") as stage_pool,
            tc.tile_pool(name="py", bufs=5, space="PSUM") as py_pool,
        ):
            # Replicated parameters, chunked so the first matmuls can start
            # as soon as their slice arrives.
            w1s = const.tile([P, DC, F], f16)  # [p, dc, f] <- w1[dc*128+p, f]
            w2s = const.tile([P, FC, D], f16)  # [p, fc, d] <- w2[fc*128+p, d]
            b1s = const.tile([P, FC], f32)  # [p, fc] <- b1[fc*128+p]
            b2s = const.tile([P, D], f32)  # b2 replicated to all partitions

            w1r = w1.rearrange("(dc p) f -> p dc f", p=P)
            w2r = w2.rearrange("(fc p) d -> p fc d", p=P)

            def load_xT(blk):
                # DMA xbar transpose straight from DRAM:
                # out[d_in, dc, r] = x[blk*512 + r, dc*128 + d_in]
                xT = xt_pool.tile([P, DC, R_BLOCK], f16, name="xT")
                rows = x[blk * R_BLOCK : (blk + 1) * R_BLOCK, :]
                dma = nc.sync.dma_start(xT[:], rows, transpose=True)
                return xT, dma

            def load_xn(blk, after=None):
                # Natural-layout load on the SWDGE (gpsimd) queue: keeps it
                # off the serialized HWDGE completion chain.
                rows = x[blk * R_BLOCK : (blk + 1) * R_BLOCK, :]
                src_ap = rows.rearrange("(rs p) d -> p rs d", p=P)
                xn = xin_pool.tile([P, RT, D], f16, name="xn")
                dma = nc.gpsimd.dma_start(xn[:], src_ap)
                if after is not None:
                    tile.add_dep_helper(
                        dma.ins, after.ins, sync=False,
                        reason="xn load ordered after paired transpose",
                    )
                return xn

            def make_mask(xn, after=None):
                # Spike mask: 1.0 where max_d |x| > threshold else 0.0.
                amax = mask_pool.tile([P, RT], f32, name="amax")
                reduce_inst = nc.vector.tensor_reduce(
                    amax[:],
                    xn[:],
                    axis=mybir.AxisListType.X,
                    op=mybir.AluOpType.max,
                    apply_absolute_value=True,
                )
                if after is not None:
                    # Scheduling-only edge: keep this block's reduce behind
                    # the named instruction in the DVE queue so the
                    # scheduler can't hoist it ahead of latency-critical
                    # DVE work (e.g. the startup xT0 copies).
                    tile.add_dep_helper(
                        reduce_inst.ins, after.ins, sync=False,
                        reason="mask reduce after critical DVE work",
                    )
                mask = mask_pool.tile([P, RT], f32, name="mask")
                nc.vector.tensor_scalar(
                    mask[:], amax[:], THRESHOLD, None, op0=mybir.AluOpType.is_gt
                )
                return mask

            # PE clock warm-up: the PE ramps to full clock only after ~3us
            # of sustained activity (HAM gate).  Burn the ramp on
            # dependency-free dummy matmuls during the startup DMA window
            # (sized to hand off into block 0's PE transposes seamlessly).
            bf16 = mybir.dt.bfloat16
            wsrc = const.tile([P, D], bf16)
            nc.vector.memset(wsrc[:, 0:P], 0.0)
            nc.vector.memset(wsrc[:, P:D], 0.0)
            wdummy = py_pool.tile([P, D], f32, name="wdummy", tag="psy")
            # Dummy activation: forces the ReLU act-table load (~1.3us)
            # to happen at t~0 instead of in front of the first real relu.
            actwarm = const.tile([P, 1], f32)
            nc.scalar.activation(
                actwarm[:], wsrc[:, 0:1], mybir.ActivationFunctionType.Relu
            )
            nc.tensor.matmul(
                wdummy[:, 0:P], wsrc[:, 0:P], wsrc[:, 0:P], start=True,
                stop=True,
            )
            for _ in range(7):
                nc.tensor.matmul(
                    wdummy[:], wsrc[:, 0:P], wsrc[:], start=True, stop=True
                )

            # --- startup.  HWDGE DMAs on one queue (SP) pipeline
            # back-to-back, but a DmaTransposeAnt acts as a completion
            # barrier on its queue (it fans out over all 16 DMA engines),
            # so the critical startup stream must not sit behind one:
            #   SP   : xn(0), w1 in 4 chunks (paced against mm1), xT(1)
            #          transpose, w2 in 4 chunks (paced against block 0's
            #          deferred mm2 phase), then per-block xT + y-stores.
            #   SWDGE: identity (host-provided), b1, xn(1), b2, then
            #          per-block xn loads for the spike mask.
            # Block 0's xT is built on the PE (f16 transpose via identity
            # matmul) from xn(0) right after the warm-up dummies, so the
            # first mm1 is gated only on xn(0) + the first w1 chunk.
            rows0 = x[0:R_BLOCK, :].rearrange("(rs p) d -> p rs d", p=P)
            xn0 = xin_pool.tile([P, RT, D], f16, name="xn")
            nc.sync.dma_start(xn0[:], rows0)
            ident = const.tile([P, P], f16)
            nc.gpsimd.dma_start(ident[:], identity[:, :])
            nc.gpsimd.dma_start(b1s[:], b1.rearrange("(p fc) -> p fc", p=P))
            nc.gpsimd.dma_start(b2s[:], b2[None, :].to_broadcast([P, D]))
            for wc in range(4):
                nc.sync.dma_start(
                    w1s[:, :, 512 * wc : 512 * (wc + 1)],
                    w1r[:, :, 512 * wc : 512 * (wc + 1)],
                )
            if n_iter > 1:
                xT1, xT1_dma = load_xT(1 % N_BLOCKS)
                xn1 = load_xn(1 % N_BLOCKS)
            else:
                xT1, xT1_dma, xn1 = None, None, None
            for wc in range(4):
                nc.sync.dma_start(
                    w2s[:, 4 * wc : 4 * (wc + 1), :],
                    w2r[:, 4 * wc : 4 * (wc + 1), :],
                )

            xT0 = xt_pool.tile([P, DC, R_BLOCK], f16, name="xT")
            last_copy = None
            for dc in range(DC):
                pt = py_pool.tile([P, RT, P], f16, name="pt", tag="psy")
                for rs in range(RT):
                    nc.tensor.transpose(
                        pt[:, rs, :],
                        xn0[:, rs, dc * P : (dc + 1) * P],
                        ident[:],
                    )
                last_copy = nc.vector.tensor_copy(xT0[:, dc, :], pt[:])
            cur = {"xT": xT0, "mask": make_mask(xn0, after=last_copy)}
            if n_iter > 1:
                nxt_pending = {"xT": xT1, "xn": xn1, "copy": last_copy}
            else:
                nxt_pending = None
            last_xT_dma = xT1_dma

            for it in range(n_iter):
                blk = it % N_BLOCKS
                xT, mask = cur["xT"], cur["mask"]

                hs = h_pool.tile([P, FC, R_BLOCK], f16, name="hs")  # h^T
                last = it == n_iter - 1 and it != 0
                psy = [
                    py_pool.tile([P, D], f32, name=f"psy{rt}", tag="psy")
                    for rt in range(RT - 1 if last else RT)
                ]
                if last:
                    # rt3 split into two independent half-width PSUM tiles
                    # so its first half's epilogue + store can run while the
                    # second half's matmul chain still owns the PE.
                    psy3h = [
                        py_pool.tile([P, D // 2], f32, name=f"psy3h{i}",
                                     tag="psy")
                        for i in range(2)
                    ]
                def mm2(fc):
                    for rt in range(RT):
                        nc.tensor.matmul(
                            psy[rt][:],
                            hs[:, fc, rt * P : (rt + 1) * P],
                            w2s[:, fc, :],
                            start=(fc == 0),
                            stop=(fc == FC - 1),
                        )

                # Software-pipelined: mm2 runs two f-chunks behind mm1/relu
                # so PE never waits on ScalarE at block boundaries.  Block 0
                # instead defers ALL mm2s past the mm1 phase so they aren't
                # stuck behind the still-streaming w2.
                mm2_lag = FC if (it == 0 or it == n_iter - 1) else 2
                for fc in range(FC):
                    ph = stage_pool.tile(
                        [P, R_BLOCK], f32, name="ph", tag="stage"
                    )
                    for dc in range(DC):
                        nc.tensor.matmul(
                            ph[:],
                            w1s[:, dc, fc * P : (fc + 1) * P],
                            xT[:, dc, :],
                            start=(dc == 0),
                            stop=(dc == DC - 1),
                        )
                    nc.scalar.activation(
                        hs[:, fc, :],
                        ph[:],
                        mybir.ActivationFunctionType.Relu,
                        bias=b1s[:, fc : fc + 1],
                    )
                    if fc >= mm2_lag:
                        mm2(fc - mm2_lag)
                    # Compute block it+1's mask mid-block; prefetch block
                    # it+2's x just after (fc==8 keeps the transposes'
                    # queue-barrier clear of the startup w2 stream in the
                    # scheduler's linearization).
                    if fc == 7 and it + 1 < n_iter:
                        nxt = {
                            "xT": nxt_pending["xT"],
                            "mask": make_mask(
                                nxt_pending["xn"],
                                after=nxt_pending.get("copy"),
                            ),
                        }
                    if fc == 8 and it + 1 < n_iter:
                        if it + 2 < n_iter:
                            xT_n, xT_n_dma = load_xT((it + 2) % N_BLOCKS)
                            xn_n = load_xn(
                                (it + 2) % N_BLOCKS, after=last_xT_dma
                            )
                            last_xT_dma = xT_n_dma
                            nxt_pending = {"xT": xT_n, "xn": xn_n}
                        else:
                            nxt_pending = None
                # b2 * mask per row-subtile (emitted after the matmul loop
                # and, for block 0, pinned behind the startup xT0 copies so
                # its wait on the b2 DMA can't poison the in-order DVE queue
                # ahead of them).
                yb = out_pool.tile([P, RT, D], f32, name="yb")
                for rt in range(RT):
                    yb_inst = nc.vector.tensor_scalar_mul(
                        yb[:, rt, :], b2s[:], mask[:, rt : rt + 1]
                    )
                    if it == 0:
                        tile.add_dep_helper(
                            yb_inst.ins, last_copy.ins, sync=False,
                            reason="yb after startup xT copies",
                        )
                # Epilogue: yout = psy*mask + b2*mask, one fused DVE op per
                # row-subtile (psy bank freed after a single op).
                yout = out_pool.tile([P, RT, D], f32, name="yout")

                def epilogue(rt):
                    nc.vector.scalar_tensor_tensor(
                        yout[:, rt, :],
                        psy[rt][:],
                        mask[:, rt : rt + 1],
                        yb[:, rt, :],
                        op0=mybir.AluOpType.mult,
                        op1=mybir.AluOpType.add,
                    )
                    out_rows = y[
                        blk * R_BLOCK + rt * P : blk * R_BLOCK + (rt + 1) * P, :
                    ]
                    nc.sync.dma_start(out_rows, yout[:, rt, :])

                if it == n_iter - 1 and it != 0:
                    # Last block: group the remaining mm2s by row-subtile so
                    # each subtile's epilogue + store overlaps the next
                    # subtile's matmuls; rt3 is additionally split into
                    # d-halves so only a half-width epilogue + store trails
                    # the final PE op before the kernel drain.
                    done = FC - mm2_lag
                    for rt in range(RT - 1):
                        for fc in range(done, FC):
                            nc.tensor.matmul(
                                psy[rt][:],
                                hs[:, fc, rt * P : (rt + 1) * P],
                                w2s[:, fc, :],
                                start=(fc == 0),
                                stop=(fc == FC - 1),
                            )
                        epilogue(rt)
                    # rt3 in d-halves on independent PSUM tiles: half 1's
                    # epilogue + store run while half 2's chain still owns
                    # the PE; only a half-width epilogue + store trails the
                    # final matmul.
                    rt = RT - 1
                    for dh in range(2):
                        dsl = slice(dh * (D // 2), (dh + 1) * (D // 2))
                        for fc in range(done, FC):
                            nc.tensor.matmul(
                                psy3h[dh][:],
                                hs[:, fc, rt * P : (rt + 1) * P],
                                w2s[:, fc, dsl],
                                start=(fc == 0),
                                stop=(fc == FC - 1),
                            )
                        nc.vector.scalar_tensor_tensor(
                            yout[:, rt, dsl],
                            psy3h[dh][:],
                            mask[:, rt : rt + 1],
                            yb[:, rt, dsl],
                            op0=mybir.AluOpType.mult,
                            op1=mybir.AluOpType.add,
                        )
                        nc.sync.dma_start(
                            y[
                                blk * R_BLOCK + rt * P : blk * R_BLOCK
                                + (rt + 1) * P,
                                dsl,
                            ],
                            yout[:, rt, dsl],
                        )
                else:
                    for fc in range(FC - mm2_lag, FC):
                        mm2(fc)
                    for rt in range(RT):
                        epilogue(rt)
                if it + 1 < n_iter:
                    cur = nxt

    nc.finalize()
    return nc


def _get_program():
    if "nc" not in _CACHE:
        _CACHE["nc"] = _build_program()
    return _CACHE["nc"]


def kernel(x, w1, b1, w2, b2, _trace=False):
    from concourse.bass_utils import run_bass_kernel_spmd

    # fp16 casts are host-side (free); accumulation on-device stays f32.
    x = np.ascontiguousarray(np.asarray(x, dtype=np.float32))
    x16 = x.astype(np.float16)
    w1h = np.ascontiguousarray(np.asarray(w1, dtype=np.float16))
    # b1 pre-arranged host-side to [p*FC+fc] so the device load is a
    # contiguous 64B-per-partition DMA instead of 2048 4-byte descriptors.
    b1 = np.asarray(b1, dtype=np.float32)
    b1p = np.ascontiguousarray(b1.reshape(FC, P).T).reshape(-1)
    w2h = np.ascontiguousarray(np.asarray(w2, dtype=np.float16))
    b2 = np.ascontiguousarray(np.asarray(b2, dtype=np.float32))

    B, T, S, Dd = x16.shape
    xf = x16.reshape(-1, Dd)
    shards = np.split(xf, N_CORES, axis=0)
    ident = np.eye(128, dtype=np.float16)
    in_maps = [
        {"x": s, "w1": w1h, "b1": b1p, "w2": w2h, "b2": b2, "identity": ident}
        for s in shards
    ]

    nc = _get_program()
    # The axon-tunneled devices occasionally throw a transient
    # NRT_EXEC_UNIT_UNRECOVERABLE; a fresh attempt succeeds.
    last_err = None
    for _attempt in range(3):
        try:
            res = run_bass_kernel_spmd(
                nc, in_maps, list(range(N_CORES)), trace=_trace
            )
            break
        except Exception as e:  # noqa: BLE001 - retry transient device faults
            last_err = e
            if "UNRECOVERABLE" not in str(e) and "UNAVAILABLE" not in str(e):
                raise
    else:
        raise last_err
    yf = np.concatenate([r["y"] for r in res.results], axis=0)
    out = yf.reshape(B, T, S, Dd).astype(np.float32)
    if _trace:
        return out, res
    return out
